# Initial kernel scaffold
#
"""DeepSeek-V3 token-choice top-k router on 8 Trainium2 NeuronCores.

Strategy (per core, data-parallel over tokens; 1024 tokens/core):
  - Host: x and gate_w.T are scaled by 4096 (exact power of 2) and split into
    fp16 hi/lo pairs (hi = rn(a), lo = rn(a - hi); 22+ mantissa bits combined,
    scaling keeps everything out of the fp16 subnormal range). x is also
    pre-transposed to d-major [128d, token] chunk layout so the device does
    ZERO transposes and zero precision-split work; hi/lo are packed adjacent
    per 7-chunk piece so one DMA descriptor fetches both.
  - Device per 128-token tile: 56 contraction chunks x 3 accumulating fp16
    matmuls (xh@wh + xh@wl + xl@wh; dropped xl@wl term is ~2^-22 relative)
    into a [128, 256] PSUM logits tile. fp16 streams the PE at 1 col/cycle
    (same as f32r) but halves gate DMA and speeds LDWEIGHTS.
  - x streams through a 12-deep piece pool on the SP HWDGE ring; gate weight
    pieces ride the ACT ring so both FIFOs pace independently. The first
    three tiles are ping-ponged at piece granularity (21 chunks of PE work
    per piece group) so the PE stays fed while the gate weight streams in;
    ~40 warmup matmuls flip the PE HAM clock gate to 2.4 GHz first.
  - ACT: sigmoid(logits * 2^-24) PSUM->SBUF (scale undone for free).
  - DVE: bias add, hardware top-8 (`max`/`max_index`) for group top-2 sums,
    top-4 group threshold, masked top-8, one-hot score gathers, normalization.
"""

import numpy as np

N = 8192
D = 7168
E = 256
G = 8
EPG = E // G  # 32
TOPK_GROUP = 4
TOP_K = 8
SCALING = 2.5
N_CORES = 8
NPC = N // N_CORES  # 1024 tokens per core
P = 128
KC = D // P  # 56 contraction chunks
TT = NPC // P  # 8 token tiles per core
GPC = 7  # chunks per piece (both x and gate-weight DMA granularity)
NGP = KC // GPC  # 8 pieces per tile
PW = 2 * GPC * P  # 1792: piece width in fp16 elems (hi 896 | lo 896)
PINGPONG = 3  # tiles interleaved at startup
WARMUP_MM = 160
SX = 4096.0  # x scale (2^12)
SW = 4096.0  # w scale (2^12)

_CACHE = {}


def build_program():
    import concourse.bacc as bacc
    import concourse.mybir as mybir
    from concourse import tile

    nc = bacc.Bacc(
        "TRN2",
        target_bir_lowering=False,
        debug=False,
        enable_asserts=False,
        num_devices=N_CORES,
    )
    f16 = mybir.dt.float16
    bf16 = mybir.dt.bfloat16
    f32 = mybir.dt.float32
    i32 = mybir.dt.int32
    u32 = mybir.dt.uint32
    AF = mybir.ActivationFunctionType
    OP = mybir.AluOpType
    AX = mybir.AxisListType

    x_d = nc.dram_tensor("x2", [P, TT * NGP * PW], f16, kind="ExternalInput").ap()
    gw_d = nc.dram_tensor("gw2", [P, KC * 2 * E], f16, kind="ExternalInput").ap()
    bias_d = nc.dram_tensor("bias", [1, E], f32, kind="ExternalInput").ap()
    idx_d = nc.dram_tensor("idx", [NPC, TOP_K], i32, kind="ExternalOutput").ap()
    w_d = nc.dram_tensor("w", [NPC, TOP_K], f32, kind="ExternalOutput").ap()

    with tile.TileContext(nc) as tc:
        with (
            tc.tile_pool(name="const", bufs=1) as const_pool,
            tc.tile_pool(name="gw", bufs=1) as gw_pool,
            tc.tile_pool(name="xp", bufs=12) as x_pool,
            tc.tile_pool(name="plog", bufs=4, space="PSUM") as plog_pool,
            tc.tile_pool(name="junk", bufs=1, space="PSUM") as junk_pool,
            tc.tile_pool(name="work", bufs=2) as work_pool,
            tc.tile_pool(name="outs", bufs=2) as out_pool,
        ):
            # ---- gate weight pieces on the ACT HWDGE ring ----
            bias_sb = const_pool.tile([1, E], f32, name="biassb")
            nc.sync.dma_start(bias_sb[:], bias_d[:])
            gw_sb = []
            q = GPC * 2 * E
            for i in range(NGP):
                gw_sb.append(gw_pool.tile([P, q], f16, name=f"gw{i}"))
                nc.scalar.dma_start(gw_sb[i][:], gw_d[:, i * q : (i + 1) * q])
            gw_v = [g[:].rearrange("p (k e) -> p k e", k=GPC) for g in gw_sb]

            # ---- x pieces on the SP ring, in exact consumption order; the
            # 12-deep pool + FIFO self-pace the prefetch.
            pieces = {}

            def fetch_piece(t, pi):
                pc = x_pool.tile([P, PW], f16, tag="xp", name=f"x{t}p{pi}")
                pieces[(t, pi)] = pc
                base = (t * NGP + pi) * PW
                nc.sync.dma_start(pc[:], x_d[:, base : base + PW])

            for i in range(NGP):
                for t in range(PINGPONG):
                    fetch_piece(t, i)
            for t in range(PINGPONG, TT):
                for i in range(NGP):
                    fetch_piece(t, i)

            # ---- constants ----
            iota_i = const_pool.tile([P, E], i32)
            nc.gpsimd.iota(iota_i[:], pattern=[[1, E]], base=0, channel_multiplier=0)
            iota_f = const_pool.tile([P, E], f32)
            nc.vector.tensor_copy(iota_f[:], iota_i[:])
            bias_rep = const_pool.tile([P, E], f32)
            nc.gpsimd.partition_broadcast(bias_rep[:], bias_sb[0:1, :])

            # ---- PE warmup: flip the HAM clock gate to 8/8 during the
            # initial DMA wait (keeps the first real matmuls at 2.4 GHz).
            ij = iota_f[:].bitcast(f16)  # [P, 512] garbage-but-finite fp16
            junk = junk_pool.tile([P, 64], f32)
            for _ in range(WARMUP_MM):
                nc.tensor.matmul(junk[:], ij[:, 0:P], ij[:, 0:64], start=True,
                                 stop=True)

            def chunk_ap(t, k):
                pc = pieces[(t, k // GPC)]
                off = (k % GPC) * P
                return pc[:, off : off + P], pc[:, GPC * P + off : GPC * P + off + P]

            def mm_chunks(t, plog, k0, k1, three=False):
                # Default: plog is [P, 2E]; one 512-wide matmul accumulates
                # xh@[wh|wl], a 256-wide one adds xl@wh into the low half; the
                # halves are summed on DVE during routing. three=True instead
                # runs 3 256-wide matmuls all into [0:E] (no fold needed) —
                # used for the last tile whose routing is the kernel tail.
                for k in range(k0, k1):
                    pc, sk = divmod(k, GPC)
                    whl = gw_v[pc][:, sk, :]
                    wh = gw_v[pc][:, sk, 0:E]
                    wl = gw_v[pc][:, sk, E : 2 * E]
                    xh_k, xl_k = chunk_ap(t, k)
                    if three:
                        nc.tensor.matmul(
                            plog[:, 0:E], xh_k, wh, start=(k == 0), stop=False,
                            skip_group_check=True,
                        )
                        nc.tensor.matmul(
                            plog[:, 0:E], xh_k, wl, start=False, stop=False,
                            skip_group_check=True,
                        )
                        nc.tensor.matmul(
                            plog[:, 0:E], xl_k, wh, start=False,
                            stop=(k == KC - 1), skip_group_check=True,
                        )
                    else:
                        nc.tensor.matmul(
                            plog[:], xh_k, whl, start=(k == 0), stop=False,
                            skip_group_check=True,
                        )
                        nc.tensor.matmul(
                            plog[:, 0:E], xl_k, wh, start=False,
                            stop=(k == KC - 1), skip_group_check=True,
                        )

            def routing(t, plog, folded=True):
                # ---- routing for this token tile ----
                scores = work_pool.tile([P, E], f32, tag="scores")
                if folded:
                    half2 = work_pool.tile([P, E], f32, tag="half2")
                    nc.scalar.copy(half2[:], plog[:, E : 2 * E])
                    lsum = work_pool.tile([P, E], f32, tag="lsum")
                    nc.vector.tensor_tensor(
                        lsum[:], plog[:, 0:E], half2[:], op=OP.add
                    )
                    srcap = lsum[:]
                else:
                    srcap = plog[:, 0:E]
                nc.scalar.activation(
                    scores[:], srcap, AF.Sigmoid, scale=1.0 / (SX * SW)
                )

                sfc = work_pool.tile([P, E], f32, tag="sfc")
                nc.vector.tensor_tensor(sfc[:], scores[:], bias_rep[:], op=OP.add)

                # per-group top-8 (need top-2 of each group of 32)
                gtops = work_pool.tile([P, G * 8], f32, tag="gtops")
                for g in range(G):
                    nc.vector.max(
                        gtops[:, g * 8 : (g + 1) * 8],
                        sfc[:, g * EPG : (g + 1) * EPG],
                    )
                gv = gtops[:].rearrange("p (g k) -> p g k", g=G)
                gs = work_pool.tile([P, G], f32, tag="gs")
                nc.vector.tensor_tensor(gs[:], gv[:, :, 0], gv[:, :, 1], op=OP.add)

                # top-4 groups -> mask
                gtop8 = work_pool.tile([P, 8], f32, tag="gtop8")
                nc.vector.max(gtop8[:], gs[:])
                gmask = work_pool.tile([P, G], f32, tag="gmask")
                nc.vector.tensor_scalar(
                    gmask[:], gs[:], gtop8[:, TOPK_GROUP - 1 : TOPK_GROUP], None,
                    op0=OP.is_ge,
                )

                # masked scores
                tmp = work_pool.tile([P, E], f32, tag="tmp")
                for g in range(G):
                    nc.vector.tensor_scalar(
                        tmp[:, g * EPG : (g + 1) * EPG],
                        sfc[:, g * EPG : (g + 1) * EPG],
                        gmask[:, g : g + 1],
                        None,
                        op0=OP.mult,
                    )

                # top-8 values + indices
                vals = work_pool.tile([P, TOP_K], f32, tag="vals")
                nc.vector.max(vals[:], tmp[:])
                idxu = work_pool.tile([P, TOP_K], u32, tag="idxu")
                nc.vector.max_index(idxu[:], vals[:], tmp[:])
                idxf = work_pool.tile([P, TOP_K], f32, tag="idxf")
                nc.vector.tensor_copy(idxf[:], idxu[:])

                # idx output is ready now — ship it while the gathers run
                idx_out = out_pool.tile([P, TOP_K], i32, tag="idxout")
                nc.vector.tensor_copy(idx_out[:], idxu[:])
                nc.scalar.dma_start(idx_d[t * P : (t + 1) * P, :], idx_out[:])

                # gather raw sigmoid scores at the selected indices
                w8 = out_pool.tile([P, TOP_K], f32, tag="w8")
                scratch = work_pool.tile([P, E], f32, tag="scratch")
                for j in range(TOP_K):
                    nc.vector.scalar_tensor_tensor(
                        scratch[:],
                        iota_f[:],
                        idxf[:, j : j + 1],
                        scores[:],
                        op0=OP.is_equal,
                        op1=OP.mult,
                        accum_out=w8[:, j : j + 1],
                    )

                # normalize + scale (wsum > 0 always: sigmoid outputs)
                wsum = work_pool.tile([P, 1], f32, tag="wsum")
                nc.vector.reduce_sum(wsum[:], w8[:], axis=AX.X)
                wrec = work_pool.tile([P, 1], f32, tag="wrec")
                nc.vector.reciprocal(wrec[:], wsum[:])
                w_out = out_pool.tile([P, TOP_K], f32, tag="wout")
                nc.vector.tensor_scalar(
                    w_out[:], w8[:], wrec[:, 0:1], float(SCALING),
                    op0=OP.mult, op1=OP.mult,
                )
                nc.scalar.dma_start(w_d[t * P : (t + 1) * P, :], w_out[:])

            # ---- tiles 0..2: ping-pong at piece granularity so PE work
            # tracks DMA arrival while the gate weight streams in.
            plogs = {
                t: plog_pool.tile([P, 2 * E], f32, tag="plog", name=f"plog{t}")
                for t in range(PINGPONG)
            }
            for i in range(NGP):
                for t in range(PINGPONG):
                    mm_chunks(t, plogs[t], i * GPC, (i + 1) * GPC)
            for t in range(PINGPONG):
                routing(t, plogs[t])

            # ---- tiles 3..7: straight pipeline (pieces prefetched by pool);
            # the last tile uses the 3-matmul form so its routing (the kernel
            # tail) skips the halves-fold.
            for t in range(PINGPONG, TT):
                plog = plog_pool.tile([P, 2 * E], f32, tag="plog", name=f"plog{t}")
                last = t == TT - 1
                mm_chunks(t, plog, 0, KC, three=last)
                routing(t, plog, folded=not last)

    nc.compile()
    return nc


def _get_nc(**kw):
    key = tuple(sorted(kw.items()))
    if key not in _CACHE:
        _CACHE[key] = build_program(**kw)
    return _CACHE[key]


def _pack_x(xh, xl):
    # [8192, 7168] fp16 hi/lo -> [8 cores, 128, TT*NGP*1792]; per core,
    # partition p holds d = k*128+p; pieces of 7 chunks with hi|lo adjacent.
    def five(a):
        # [c, t, j, k, p] -> [c, p, t, k, j] -> split k into (pi, ks)
        b = a.reshape(N_CORES, TT, P, KC, P).transpose(0, 4, 1, 3, 2)
        return b.reshape(N_CORES, P, TT, NGP, GPC, P)

    comb = np.stack([five(xh), five(xl)], axis=4)  # [c,p,t,pi,{h,l},ks,j]
    return np.ascontiguousarray(comb).reshape(N_CORES, P, TT * NGP * PW)


def _prep_inputs(x, gate_w, bias):
    xs = x * np.float32(SX)
    xh = xs.astype(np.float16)
    xl = (xs - xh.astype(np.float32)).astype(np.float16)
    xp = _pack_x(xh, xl)

    ws = np.ascontiguousarray(gate_w.T) * np.float32(SW)  # [D, E]
    wh = ws.astype(np.float16)
    wl = (ws - wh.astype(np.float32)).astype(np.float16)
    gw2 = np.concatenate(
        [wh.reshape(KC, P, E), wl.reshape(KC, P, E)], axis=2
    )  # [KC, P, 2E]
    gw2 = np.ascontiguousarray(gw2.transpose(1, 0, 2)).reshape(P, KC * 2 * E)
    bias2d = np.ascontiguousarray(bias.reshape(1, E))
    return xp, gw2, bias2d


def _run(x, gate_w, bias, trace=False, **build_kw):
    from concourse.bass_utils import run_bass_kernel_spmd

    x = np.ascontiguousarray(np.asarray(x, dtype=np.float32))
    gate_w = np.ascontiguousarray(np.asarray(gate_w, dtype=np.float32))
    bias = np.ascontiguousarray(np.asarray(bias, dtype=np.float32))
    nc = _get_nc(**build_kw)
    xp, gw2, bias2d = _prep_inputs(x, gate_w, bias)
    in_maps = [
        {"x2": xp[c], "gw2": gw2, "bias": bias2d} for c in range(N_CORES)
    ]
    res = run_bass_kernel_spmd(nc, in_maps, core_ids=list(range(N_CORES)), trace=trace)
    idx = np.concatenate([res.results[c]["idx"] for c in range(N_CORES)], axis=0)
    w = np.concatenate([res.results[c]["w"] for c in range(N_CORES)], axis=0)
    return (idx.astype(np.int32), w.astype(np.float32)), res


def kernel(x, gate_w, bias):
    (idx, w), _ = _run(x, gate_w, bias)
    return idx, w



# revision 16
# speedup vs baseline: 2.2242x; 2.2242x over previous
"""DeepSeek-V3 token-choice top-k router on 8 Trainium2 NeuronCores.

Strategy (per core, data-parallel over tokens; 1024 tokens/core):
  - Host: x and gate_w.T are scaled by 4096 and cast to a SINGLE fp16
    copy (the PE computes fp16 matmuls at ~bf16 internal precision, so
    hi/lo splitting buys no accuracy on HW — one pass is 3x cheaper).
    x is pre-transposed to d-major [128d, token] chunk layout.
  - Device per 128-token tile: 56 contraction chunks x 1 fp16 matmul
    (N=256 streaming columns) accumulating into a [128, 256] PSUM
    logits tile. x chunks are the stationary operand (LDWEIGHTS), the
    gate weight streams; FWL halves the LDWEIGHTS cost for fp16.
  - x streams per-tile (2 pieces/tile) on the SP HWDGE ring; the gate
    weight rides the ACT ring in 4 pieces so the first tile's matmuls
    start as soon as the first chunks land. ~40 warmup matmuls flip
    the PE HAM clock gate to 2.4 GHz during the initial DMA wait.
  - ACT: sigmoid(logits * 2^-24) PSUM->SBUF (scale undone for free).
  - DVE: bias add, per-group top-8 (`max`), top-4 group threshold via
    broadcast-AP mask multiply, masked top-8 + `max_index`, fp16
    one-hot score gathers (2x DVE rate), normalization.
"""

import numpy as np

N = 8192
D = 7168
E = 256
G = 8
EPG = E // G  # 32
TOPK_GROUP = 4
TOP_K = 8
SCALING = 2.5
N_CORES = 8
NPC = N // N_CORES  # 1024 tokens per core
P = 128
KC = D // P  # 56 contraction chunks
TT = NPC // P  # 8 token tiles per core
XPT = 2  # x pieces per tile
KPP = KC // XPT  # 28 chunks per x piece
GWP = 4  # gate-weight pieces
KPG = KC // GWP  # 14 chunks per gw piece
WARMUP_MM = 40
SX = 4096.0  # x scale (2^12)
SW = 4096.0  # w scale (2^12)

_CACHE = {}


def build_program():
    import concourse.bacc as bacc
    import concourse.mybir as mybir
    from concourse import tile

    nc = bacc.Bacc(
        "TRN2",
        target_bir_lowering=False,
        debug=False,
        enable_asserts=False,
        num_devices=N_CORES,
    )
    f16 = mybir.dt.float16
    f32 = mybir.dt.float32
    i16 = mybir.dt.int16
    i32 = mybir.dt.int32
    u32 = mybir.dt.uint32
    AF = mybir.ActivationFunctionType
    OP = mybir.AluOpType
    AX = mybir.AxisListType

    x_d = nc.dram_tensor("x2", [P, TT * KC * P], f16, kind="ExternalInput").ap()
    gw_d = nc.dram_tensor("gw2", [P, KC * E], f16, kind="ExternalInput").ap()
    bias_d = nc.dram_tensor("bias", [1, E], f32, kind="ExternalInput").ap()
    idx_d = nc.dram_tensor("idx", [NPC, TOP_K], i32, kind="ExternalOutput").ap()
    w_d = nc.dram_tensor("w", [NPC, TOP_K], f32, kind="ExternalOutput").ap()

    with tile.TileContext(nc) as tc:
        with (
            tc.tile_pool(name="const", bufs=1) as const_pool,
            tc.tile_pool(name="gw", bufs=1) as gw_pool,
            tc.tile_pool(name="xp", bufs=6) as x_pool,
            tc.tile_pool(name="plog", bufs=6, space="PSUM") as plog_pool,
            tc.tile_pool(name="junk", bufs=1, space="PSUM") as junk_pool,
            tc.tile_pool(name="work", bufs=5) as work_pool,
            tc.tile_pool(name="outs", bufs=5) as out_pool,
        ):
            # ---- gate weight pieces on the ACT HWDGE ring ----
            bias_sb = const_pool.tile([1, E], f32, name="biassb")
            nc.sync.dma_start(bias_sb[:], bias_d[:])
            gw_sb = []
            q = KPG * E
            for i in range(GWP):
                gw_sb.append(gw_pool.tile([P, q], f16, name=f"gw{i}"))
                nc.scalar.dma_start(gw_sb[i][:], gw_d[:, i * q : (i + 1) * q])
            gw_v = [g[:].rearrange("p (k e) -> p k e", k=KPG) for g in gw_sb]

            # ---- x pieces on the SP ring, in consumption order; the pool
            # + FIFO self-pace the prefetch.
            pieces = {}
            for t in range(TT):
                for i in range(XPT):
                    pc = x_pool.tile([P, KPP * P], f16, tag="xp", name=f"x{t}p{i}")
                    pieces[(t, i)] = pc
                    base = (t * KC + i * KPP) * P
                    nc.sync.dma_start(pc[:], x_d[:, base : base + KPP * P])

            # ---- constants ----
            iota_i = const_pool.tile([P, E], i32)
            nc.gpsimd.iota(iota_i[:], pattern=[[1, E]], base=0, channel_multiplier=0)
            iota_f = const_pool.tile([P, E], f32)
            nc.vector.tensor_copy(iota_f[:], iota_i[:])
            bias_rep = const_pool.tile([P, E], f32)
            nc.gpsimd.partition_broadcast(bias_rep[:], bias_sb[0:1, :])
            # ranks 1..8 (int16) — scatter payload for the rank map
            ranks16 = const_pool.tile([P, TOP_K], i16)
            nc.gpsimd.iota(ranks16[:], pattern=[[1, TOP_K]], base=1,
                           channel_multiplier=0)

            # ---- PE warmup: flip the HAM clock gate toward 8/8 during the
            # initial DMA wait.
            ij = iota_f[:].bitcast(f16)[:, 0:E]  # garbage-but-finite fp16
            junk = junk_pool.tile([P, 64], f32)
            for _ in range(WARMUP_MM):
                nc.tensor.matmul(junk[:], ij[:, 0:P], ij[:, 0:64], start=True,
                                 stop=True)

            def mm_tile(t, plog):
                for k in range(KC):
                    xp = pieces[(t, k // KPP)]
                    xk = xp[:, (k % KPP) * P : (k % KPP + 1) * P]
                    wk = gw_v[k // KPG][:, k % KPG, :]
                    nc.tensor.matmul(
                        plog[:], xk, wk, start=(k == 0), stop=(k == KC - 1),
                        skip_group_check=True,
                    )

            def routing(t, plog):
                scores = work_pool.tile([P, E], f32, tag="scores")
                nc.scalar.activation(
                    scores[:], plog[:], AF.Sigmoid, scale=1.0 / (SX * SW)
                )
                # fp16 copy (ACT engine) — scatter payload for the gather
                scores16 = work_pool.tile([P, E], f16, tag="scores16")
                nc.scalar.copy(scores16[:], scores[:])

                sfc = work_pool.tile([P, E], f32, tag="sfc")
                nc.vector.tensor_tensor(sfc[:], scores[:], bias_rep[:], op=OP.add)

                # per-group top-8 (need top-2 of each group of 32)
                gtops = work_pool.tile([P, G * 8], f32, tag="gtops")
                for g in range(G):
                    nc.vector.max(
                        gtops[:, g * 8 : (g + 1) * 8],
                        sfc[:, g * EPG : (g + 1) * EPG],
                    )
                gv = gtops[:].rearrange("p (g k) -> p g k", g=G)
                gs = work_pool.tile([P, G], f32, tag="gs")
                nc.vector.tensor_tensor(gs[:], gv[:, :, 0], gv[:, :, 1], op=OP.add)

                # top-4 groups -> mask
                gtop8 = work_pool.tile([P, 8], f32, tag="gtop8")
                nc.vector.max(gtop8[:], gs[:])
                gmask = work_pool.tile([P, G], f32, tag="gmask")
                nc.vector.tensor_scalar(
                    gmask[:], gs[:], gtop8[:, TOPK_GROUP - 1 : TOPK_GROUP], None,
                    op0=OP.is_ge,
                )

                # masked scores (broadcast the group mask over the 32 experts
                # of each group with a stride-0 AP)
                tmp = work_pool.tile([P, E], f32, tag="tmp")
                sfc_g = sfc[:].rearrange("p (g e) -> p g e", g=G)
                tmp_g = tmp[:].rearrange("p (g e) -> p g e", g=G)
                gmask_b = gmask[:].rearrange("p (g o) -> p g o", o=1).broadcast_to(
                    [P, G, EPG]
                )
                nc.vector.tensor_tensor(tmp_g, sfc_g, gmask_b, op=OP.mult)

                # top-8 values + indices
                vals = work_pool.tile([P, TOP_K], f32, tag="vals")
                nc.vector.max(vals[:], tmp[:])
                idxu = work_pool.tile([P, TOP_K], u32, tag="idxu")
                nc.vector.max_index(idxu[:], vals[:], tmp[:])

                # idx output is ready now -- ship it while the gather runs
                nc.scalar.dma_start(
                    idx_d[t * P : (t + 1) * P, :], idxu[:].bitcast(i32)
                )

                # gather scores[idx] via two gpsimd local_scatters:
                #   1) rank_map[e] = slot+1 at selected experts (0 elsewhere)
                #   2) w16[slot] = scores16[e] scattered by rank_map-1
                idx16 = work_pool.tile([P, TOP_K], i16, tag="idx16")
                nc.vector.tensor_copy(idx16[:], idxu[:])
                rank_map = work_pool.tile([P, E], i16, tag="rankmap")
                nc.gpsimd.local_scatter(
                    rank_map[:], ranks16[:], idx16[:],
                    channels=P, num_elems=E, num_idxs=TOP_K,
                )
                targ = work_pool.tile([P, E], i16, tag="targ")
                nc.vector.tensor_scalar(
                    targ[:], rank_map[:], 1, None, op0=OP.subtract
                )
                w16 = work_pool.tile([P, TOP_K], f16, tag="w16")
                nc.gpsimd.local_scatter(
                    w16[:], scores16[:], targ[:],
                    channels=P, num_elems=TOP_K, num_idxs=E,
                )
                w8 = out_pool.tile([P, TOP_K], f32, tag="w8")
                nc.vector.tensor_copy(w8[:], w16[:])

                # normalize + scale (wsum > 0 always: sigmoid outputs)
                wsum = work_pool.tile([P, 1], f32, tag="wsum")
                nc.vector.reduce_sum(wsum[:], w8[:], axis=AX.X)
                wrec = work_pool.tile([P, 1], f32, tag="wrec")
                nc.vector.reciprocal(wrec[:], wsum[:])
                w_out = out_pool.tile([P, TOP_K], f32, tag="wout")
                nc.vector.tensor_scalar(
                    w_out[:], w8[:], wrec[:, 0:1], float(SCALING),
                    op0=OP.mult, op1=OP.mult,
                )
                nc.scalar.dma_start(w_d[t * P : (t + 1) * P, :], w_out[:])

            for t in range(TT):
                plog = plog_pool.tile([P, E], f32, tag="plog", name=f"plog{t}")
                mm_tile(t, plog)
                routing(t, plog)

    nc.compile()
    return nc


def _get_nc(**kw):
    key = tuple(sorted(kw.items()))
    if key not in _CACHE:
        _CACHE[key] = build_program(**kw)
    return _CACHE[key]


def _pack_x(xh):
    # [8192, 7168] fp16 -> [8 cores, 128, TT*KC*128]; per core, partition p
    # holds d = k*128+p; free index = t*(KC*128) + k*128 + token.
    b = xh.reshape(N_CORES, TT, P, KC, P).transpose(0, 4, 1, 3, 2)
    return np.ascontiguousarray(b).reshape(N_CORES, P, TT * KC * P)


def _prep_inputs(x, gate_w, bias):
    xh = (x * np.float32(SX)).astype(np.float16)
    xp = _pack_x(xh)

    ws = np.ascontiguousarray(gate_w.T) * np.float32(SW)  # [D, E]
    wh = ws.astype(np.float16)
    gw2 = np.ascontiguousarray(
        wh.reshape(KC, P, E).transpose(1, 0, 2)
    ).reshape(P, KC * E)
    bias2d = np.ascontiguousarray(bias.reshape(1, E))
    return xp, gw2, bias2d


def _run(x, gate_w, bias, trace=False, **build_kw):
    from concourse.bass_utils import run_bass_kernel_spmd

    x = np.ascontiguousarray(np.asarray(x, dtype=np.float32))
    gate_w = np.ascontiguousarray(np.asarray(gate_w, dtype=np.float32))
    bias = np.ascontiguousarray(np.asarray(bias, dtype=np.float32))
    nc = _get_nc(**build_kw)
    xp, gw2, bias2d = _prep_inputs(x, gate_w, bias)
    in_maps = [
        {"x2": xp[c], "gw2": gw2, "bias": bias2d} for c in range(N_CORES)
    ]
    res = run_bass_kernel_spmd(nc, in_maps, core_ids=list(range(N_CORES)), trace=trace)
    idx = np.concatenate([res.results[c]["idx"] for c in range(N_CORES)], axis=0)
    w = np.concatenate([res.results[c]["w"] for c in range(N_CORES)], axis=0)
    return (idx.astype(np.int32), w.astype(np.float32)), res


def kernel(x, gate_w, bias):
    (idx, w), _ = _run(x, gate_w, bias)
    return idx, w
